# revision 2
# baseline (speedup 1.0000x reference)
"""MoE routed dynamics kernel for Trainium2 (8 NeuronCores, expert-parallel).

Problem: for each row b of a [B, D+A] input, route through one of P=8
two-layer MLPs selected by policy_indices[b]:
    h = relu(x @ W1[p] + b1[p]);  y = h @ W2[p] + b2[p]

Sharding: expert-parallel. Core p owns expert p's weights and processes
the rows routed to expert p. The all-to-all dispatch keyed on
policy_indices happens on the host at shard time (gather rows by expert,
pad to a common capacity C multiple of 8), and the inverse scatter at
unshard time.

Design (v2, all-bf16):
- Everything streams as bf16: x, W1, W2, h, y. PE streams 1 col/cycle
  for bf16 (same as fp32r), but DMA bytes halve and the DVE/ACT upcast
  stage disappears from the startup critical path. PSUM accumulation
  stays fp32; measured end-to-end rel err ~4e-3 (gate 2e-2).
- Startup: the first real matmul needs x chunk0 + W1 m-block 0 only.
  x0 rides the sync queue first; W1 m0 + biases ride scalar; W1 m1..m7
  ride gpsimd (SWDGE) as 7 per-block transfers that arrive just ahead
  of the L1 m-loop consuming them; x1..x4 + W2 drain on sync behind x0.
- W1 is m-block-major in DRAM ([128, mh, KC, 128]) so each m-block is
  one descriptor per partition.
- Warmup matmuls (bf16, N=256) on a zeroed tile ramp the PE clock
  (HAM un-throttles after ~3.4us of activity) while x0/W1m0 fly.
- L1 phase computes h = relu(W1.T x + b1) for all chunks; all h tiles
  stay resident in SBUF (~34KB/partition). L2 runs chunks in reverse
  so the kernel tail drains the small lead-in chunk, whose d-groups are
  flushed early so the final store is a single small transfer.
"""

import math

import numpy as np
import ml_dtypes

_B = 16384
_P = 8
_D = 512
_A = 64
_H = 1024
_DA = _D + _A   # 576
_KC = 5         # K chunks over DA padded to 5*128=640
_N_CORES = 8
_MH = _H // 128  # 8 L1 output groups
_MD = _D // 128  # 4 L2 output groups

_WARMUP_N = 256
_WARMUPS = 8

_kernel_cache: dict = {}


def _chunks(C: int):
    """Column chunking: 256-wide lead-in (small first x transfer, so the
    PE starts sooner), then 512-wide steady chunks and a tail."""
    assert C >= 256, C
    if C <= 512:
        return [C]
    out = [256]
    rem = C - 256
    while rem > 512:
        out.append(512)
        rem -= 512
    out.append(rem)
    return out


def _build_bass(C: int):
    import concourse.bacc as bacc
    import concourse.mybir as mybir
    from concourse.tile import TileContext

    fp32 = mybir.dt.float32
    bf16 = mybir.dt.bfloat16
    act = mybir.ActivationFunctionType

    widths = _chunks(C)
    offsets = [sum(widths[:i]) for i in range(len(widths))]
    mh, md = _MH, _MD

    nc = bacc.Bacc()
    xd = nc.declare_dram_parameter("xq", [128, _KC * C], bf16, isOutput=False)
    w1d = nc.declare_dram_parameter("w1q", [128, mh, _KC * 128], bf16, isOutput=False)
    w2d = nc.declare_dram_parameter("w2q", [128, mh, _D], bf16, isOutput=False)
    bd = nc.declare_dram_parameter("bq", [128, mh + md], fp32, isOutput=False)
    od = nc.declare_dram_parameter("oq", [128, md, C], bf16, isOutput=True)

    with TileContext(nc) as tc:
        with (
            tc.tile_pool(name="wpool", bufs=1) as wpool,
            tc.tile_pool(name="xpool", bufs=len(widths)) as xpool,
            tc.tile_pool(name="hpool", bufs=1) as hpool,
            tc.tile_pool(name="ypool", bufs=2) as ypool,
            tc.tile_pool(name="psum", bufs=8, space="PSUM") as psp,
        ):
            w1_sb = wpool.tile([128, mh, _KC, 128], bf16, tag="w1")
            w2_sb = wpool.tile([128, mh, _D], bf16, tag="w2")
            b_sb = wpool.tile([128, mh + md], fp32, tag="b")
            warm_sb = wpool.tile([128, 128 + _WARMUP_N], bf16, tag="warm")

            x_sb = [
                xpool.tile([128, _KC, nl], bf16, tag="x", name=f"x{ci}")
                for ci, nl in enumerate(widths)
            ]

            # Load schedule: x0 first on sync (HWDGE), W1 m-block0 +
            # biases on scalar, W1 m1..m7 on gpsimd in consumption
            # order, x1..x4 + W2 behind x0 on sync. Queues drain FIFO,
            # so emission order is arrival priority.
            nc.vector.memset(warm_sb[:, :], 0.0)
            nc.sync.dma_start(out=x_sb[0][:, :, :], in_=xd[:, 0 : _KC * widths[0]])
            nc.scalar.dma_start(out=w1_sb[:, 0, :, :], in_=w1d[:, 0, :])
            nc.scalar.dma_start(out=b_sb[:, :], in_=bd[:, :])
            for m in range(1, mh):
                nc.gpsimd.dma_start(out=w1_sb[:, m, :, :], in_=w1d[:, m, :])
            for ci in range(1, len(widths)):
                nc.sync.dma_start(
                    out=x_sb[ci][:, :, :],
                    in_=xd[:, _KC * offsets[ci] : _KC * (offsets[ci] + widths[ci])],
                )
            nc.sync.dma_start(out=w2_sb[:, :, 0:256], in_=w2d[:, :, 0:256])
            nc.sync.dma_start(out=w2_sb[:, :, 256:_D], in_=w2d[:, :, 256:_D])

            # PE warmup: the clock gate holds the PE at reduced rate
            # until ~3.4us of sustained activity; burn the x0/W1m0 DMA
            # wait ramping it.
            for _ in range(_WARMUPS):
                wp = psp.tile([128, _WARMUP_N], fp32, tag="ps", name="warmps")
                nc.tensor.matmul(
                    wp[:, :], warm_sb[:, 0:128], warm_sb[:, 128 : 128 + _WARMUP_N],
                    start=True, stop=True,
                )

            # Phase 1: L1 (h = relu(W1.T x + b1)) for every chunk. All h
            # tiles stay resident in SBUF.
            h_sb: dict = {}
            for ci, nl in enumerate(widths):
                x = x_sb[ci]
                for m in range(mh):
                    ps = psp.tile([128, nl], fp32, tag="ps", name=f"ps1_{ci}_{m}")
                    for k in range(_KC):
                        nc.tensor.matmul(
                            ps[:, :],
                            w1_sb[:, m, k, :],
                            x[:, k, :],
                            start=(k == 0),
                            stop=(k == _KC - 1),
                        )
                    ht = hpool.tile(
                        [128, nl], bf16, tag=f"h_{ci}_{m}", name=f"h_{ci}_{m}"
                    )
                    nc.scalar.activation(
                        ht[:, :], ps[:, :], act.Relu, bias=b_sb[:, m : m + 1]
                    )
                    h_sb[(ci, m)] = ht

            # Phase 2: L2 (y = W2.T h + b2), chunks in reverse order so
            # the kernel tail drains the small lead-in chunk.
            for ci in reversed(range(len(widths))):
                nl, n0 = widths[ci], offsets[ci]
                yt = ypool.tile([128, md, nl], bf16, tag="y", name=f"y_{ci}")
                for d in range(md):
                    ps = psp.tile([128, nl], fp32, tag="ps", name=f"ps2_{ci}_{d}")
                    for m in range(mh):
                        nc.tensor.matmul(
                            ps[:, :],
                            w2_sb[:, m, d * 128 : (d + 1) * 128],
                            h_sb[(ci, m)][:, :],
                            start=(m == 0),
                            stop=(m == mh - 1),
                        )
                    nc.vector.tensor_scalar_add(
                        yt[:, d, :], ps[:, :], b_sb[:, mh + d : mh + d + 1]
                    )
                    if ci == 0 and d == md - 2:
                        # Final chunk: flush d0..d2 early so the kernel
                        # tail is a single small d-group store.
                        nc.sync.dma_start(
                            out=od[:, 0 : md - 1, n0 : n0 + nl],
                            in_=yt[:, 0 : md - 1, :],
                        )
                if ci == 0:
                    nc.sync.dma_start(
                        out=od[:, md - 1 :, n0 : n0 + nl], in_=yt[:, md - 1 :, :]
                    )
                else:
                    nc.sync.dma_start(out=od[:, :, n0 : n0 + nl], in_=yt[:, :, :])

    nc.compile()
    return nc


def _get_bass(C: int):
    nc = _kernel_cache.get(C)
    if nc is None:
        nc = _build_bass(C)
        _kernel_cache[C] = nc
    return nc


def _prepare_in_maps(latents, actions, policy_indices, W1, b1, W2, b2):
    """Expert-parallel dispatch: returns (in_maps, C, order, offs, counts)."""
    latents = np.asarray(latents, dtype=np.float32)
    actions = np.asarray(actions, dtype=np.float32)
    pi = np.asarray(policy_indices).astype(np.int64)
    W1 = np.asarray(W1, dtype=np.float32)
    b1 = np.asarray(b1, dtype=np.float32)
    W2 = np.asarray(W2, dtype=np.float32)
    b2 = np.asarray(b2, dtype=np.float32)

    B = latents.shape[0]
    counts = np.bincount(pi, minlength=_P)
    order = np.argsort(pi, kind="stable")
    offs = np.concatenate(([0], np.cumsum(counts)))

    # Exact capacity (rounded to 8 cols): matmul free size has no
    # 128-alignment requirement, and every padded column costs 72
    # PE passes.
    C = max(256, int(math.ceil(counts.max() / 8)) * 8)

    x = np.empty((B, _DA), dtype=np.float32)
    x[:, :_D] = latents
    x[:, _D:] = actions
    x_sorted = x[order]

    mh, md = _MH, _MD
    widths = _chunks(C)
    noff = [sum(widths[:i]) for i in range(len(widths))]
    in_maps = []
    for p in range(_P):
        xp = np.zeros((C, _KC * 128), dtype=np.float32)
        xp[: counts[p], :_DA] = x_sorted[offs[p] : offs[p + 1]]
        # xq is chunk-major: chunk ci occupies flat cols
        # [KC*n0, KC*(n0+nl)) contiguously per partition, so each chunk
        # load is ONE descriptor per partition.
        xr = xp.T.reshape(_KC, 128, C).transpose(1, 0, 2).astype(ml_dtypes.bfloat16)
        xq = np.empty((128, _KC * C), dtype=ml_dtypes.bfloat16)
        for ci, nl in enumerate(widths):
            n0 = noff[ci]
            xq[:, _KC * n0 : _KC * (n0 + nl)] = xr[:, :, n0 : n0 + nl].reshape(
                128, _KC * nl
            )
        w1p = np.zeros((_KC * 128, _H), dtype=np.float32)
        w1p[:_DA] = W1[p]
        # m-block-major: [128, mh, KC, 128] so each m-block is one
        # contiguous 1280B segment per partition.
        w1q = np.ascontiguousarray(
            w1p.reshape(_KC, 128, mh, 128)
            .transpose(1, 2, 0, 3)
            .reshape(128, mh, _KC * 128)
        ).astype(ml_dtypes.bfloat16)
        w2q = (
            np.ascontiguousarray(W2[p].reshape(mh, 128, _D).transpose(1, 0, 2))
            .astype(ml_dtypes.bfloat16)
        )
        bq = np.empty((128, mh + md), dtype=np.float32)
        bq[:, :mh] = b1[p].reshape(mh, 128).T
        bq[:, mh:] = b2[p].reshape(md, 128).T
        in_maps.append({"xq": xq, "w1q": w1q, "w2q": w2q, "bq": bq})
    return in_maps, C, order, offs, counts


def kernel(latents, actions, policy_indices, W1, b1, W2, b2):
    from concourse.bass_utils import run_bass_kernel_spmd

    in_maps, C, order, offs, counts = _prepare_in_maps(
        latents, actions, policy_indices, W1, b1, W2, b2
    )
    nc = _get_bass(C)
    results = run_bass_kernel_spmd(nc, in_maps, list(range(_N_CORES))).results

    B = np.asarray(latents).shape[0]
    out = np.empty((B, _D), dtype=np.float32)
    for p in range(_P):
        oq = np.asarray(results[p]["oq"])  # [128, 4, C] bf16
        yT = oq.transpose(1, 0, 2).reshape(_D, C)
        out[order[offs[p] : offs[p + 1]]] = yT[:, : counts[p]].T.astype(np.float32)
    return out
